# revision 32
# baseline (speedup 1.0000x reference)
"""GAT (2-layer, global-softmax attention) Trainium2 Bass kernel, 8-core SPMD.

Sharding: core c owns batch c//4 and DEST-node block i0 = 128*(c%4). Each
core computes e[i_shard, j] for its 128 attention rows against all N=512
source nodes, the masked exp, and its own output rows
U[i_shard, m] = sum_j E[i,j] h[j, m] — the aggregation needs NO collective.
The only cross-core data is one AllGather (4-core group) per layer
boundary, carrying the RAW aggregation transpose U1^T plus the
softmax-denominator partial in the last payload row. Everything downstream
is linear in U1, so w1, a1I, a1J, and the biases are host-folded
(wsi2 = w1@a1I~, csj2 = b1@a1J~ + a1b~, ...) and the 1/S1 scale rides the
activation `scale` operand — no normalize step on the critical path.
Layer 2's denominator partials go back to the host, which sums and
divides during unsharding — no second collective.

Edge scores: with z = relu(s_i[i,k] + s_j[j,k] + b[k]),
e[i,j] = sum_k z[i,j,k]*a2[k]. |a2[k]| is folded into the projections
(a2*relu(x) = sign(a2)*relu(|a2|*x)) and k sorted positive-signs-first.
Per k a rank-2 TensorE matmul ([siT_k; 1]^T @ [1; sjT_k]) produces a
(128,512) f32 slab in PSUM, consumed by two parallel paths:
  S-path: ScalarE relu on slab PAIRS -> bf16 tree tiles, folded
          incrementally by wide contiguous bf16 adds (DVE 2x mode, with
          GpSimd folding alternate tiles);
  V-path: DVE scalar_tensor_tensor acc = relu(z) + acc (f32).
Sign groups accumulate separately and combine as pos - neg. All
transposes use the XBAR DMA-transpose engine (SBUF->SBUF, zero engine
cost).
"""

import sys

if "/opt/trn_rl_repo" not in sys.path:
    sys.path.insert(0, "/opt/trn_rl_repo")

import numpy as np
import ml_dtypes

import concourse.bass as bass
import concourse.mybir as mybir
import concourse.tile as tile
from concourse import bacc
from concourse.bass_utils import run_bass_kernel_spmd

BF16 = mybir.dt.bfloat16
F32 = mybir.dt.float32
FP8 = mybir.dt.float8e4
DR = mybir.MatmulPerfMode.DoubleRow
AF = mybir.ActivationFunctionType
ALU = mybir.AluOpType

B, N, IN_DIM, MEM, HID = 2, 512, 512, 300, 64
P = 128  # i-shard rows per core
NCORES = 8
GROUPS = [[0, 1, 2, 3], [4, 5, 6, 7]]
NEG_SLOPE = 0.01
MASK_OFF = 30.0  # masked logits get exp(x*0 - 30) ~ 9e-14 instead of exp(-1e30)

MC = [128, 128, 44]  # chunks of MEM=300
NJC = N // P  # 4 j-blocks

# consume mix per layer: slabs to Scalar relu + tree folds vs DVE STT.
X_S, X_V = 42, 22


def _consume_assignment(p_pos):
    """Split k in [0,64) (pos sign first) into per-engine lists and an
    interleaved unit schedule: ('S', sign, k0, k1) pairs, ('V', sign, k)."""
    units_by_sign = []
    for sign, ks in ((1, list(range(p_pos))), (0, list(range(p_pos, HID)))):
        n = len(ks)
        s_n = min(n, int(round(X_S * n / HID / 2.0)) * 2)
        v_n = n - s_n
        su = [("S", sign, ks[2 * t], ks[2 * t + 1]) for t in range(s_n // 2)]
        vu = [("V", sign, k) for k in ks[s_n:]]
        merged = []
        iters = [su, vu]
        tot = sum(len(x) for x in iters)
        idx = [0.0, 0.0]
        for _ in range(tot):
            best = max(
                (0, 1),
                key=lambda q: (len(iters[q]) - idx[q]) / max(len(iters[q]), 1),
            )
            merged.append(iters[best][int(idx[best])])
            idx[best] += 1
        units_by_sign.append((merged, s_n, n - s_n))
    (mu0, s0, v0), (mu1, s1, v1) = units_by_sign
    merged = []
    i0 = i1 = 0
    t0, t1 = len(mu0), len(mu1)
    while i0 < t0 or i1 < t1:
        if i1 >= t1 or (i0 < t0 and i0 * t1 <= i1 * t0):
            merged.append(mu0[i0])
            i0 += 1
        else:
            merged.append(mu1[i1])
            i1 += 1
    counts = {"s_pos": s0, "v_pos": v0, "s_neg": s1, "v_neg": v1}
    return merged, counts


def _tree_fold(nc, eng, tile_, nslab):
    """Fold nslab bf16 slabs (contiguous [128, nslab, 512]) down to slab 0
    with wide contiguous adds on `eng`. Returns AP of the folded slab."""
    n = nslab
    while n > 1:
        lo = n - n // 2
        w = n - lo
        eng.tensor_add(
            tile_[:, 0:w, :], tile_[:, 0:w, :], tile_[:, lo : lo + w, :]
        )
        n = lo
    return tile_[:, 0, :]


def _emit_scores(nc, pools, cst, lay, lhsJ, rhsA, sched, counts, a2b, adjt):
    """Produce + consume the 64 score slabs; epilogue to E=exp + rowsums.
    Returns (E bf16 [128,512], sE f32 [128,1])."""
    work, zp2, zp1 = pools["work"], pools["zp2"], pools["zp1"]

    tree = {}
    fill = {}
    folded = {0: [], 1: []}
    for sign in (0, 1):
        ns = counts["s_pos" if sign else "s_neg"]
        tiles = []
        for t in range((ns + 7) // 8):
            cap = min(8, ns - 8 * t)
            tiles.append(
                (
                    work.tile(
                        [128, 8, 512], BF16, tag=f"tr{sign}{t}",
                        name=f"tr{sign}{t}_{lay}",
                    ),
                    cap,
                )
            )
        tree[sign] = tiles
        fill[sign] = 0
    accs = {}

    def acc_consume(sign, zslab):
        key = ("V", sign)
        if key not in accs:
            at = work.tile(
                [128, 512], F32, tag=f"accV{sign}", name=f"accV{sign}_{lay}"
            )
            accs[key] = at
            nc.vector.memset(at[:, :], 0.0)
        at = accs[key]
        nc.vector.scalar_tensor_tensor(
            at[:, :], zslab, 0.0, at[:, :], op0=ALU.max, op1=ALU.add
        )

    def maybe_fold(sign):
        """Tile-0 of each sign folds on GpSimd with fine-grained pair adds
        that chase the relu stream; other tiles fold on DVE when complete."""
        pos = fill[sign]
        t = (pos - 1) // 8
        tt, cap = tree[sign][t]
        if t <= 1 and cap == 8:
            sl = ((pos - 1) % 8) - 1  # first slot of the pair just written
            if sl == 2:
                nc.gpsimd.tensor_add(tt[:, 0, :], tt[:, 0, :], tt[:, 1, :])
            if sl >= 2:
                nc.gpsimd.tensor_add(
                    tt[:, sl, :], tt[:, sl, :], tt[:, sl + 1, :]
                )
            if pos == t * 8 + 8:  # complete: combine pair sums on DVE
                nc.vector.tensor_add(tt[:, 0, :], tt[:, 0, :], tt[:, 2, :])
                nc.vector.tensor_add(tt[:, 4, :], tt[:, 4, :], tt[:, 6, :])
                nc.vector.tensor_add(tt[:, 0, :], tt[:, 0, :], tt[:, 4, :])
                folded[sign].append(tt[:, 0, :])
        elif pos == t * 8 + cap:
            folded[sign].append(_tree_fold(nc, nc.vector, tt, cap))

    for unit in sched:
        path, sign = unit[0], unit[1]
        if path == "S":
            k0, k1 = unit[2], unit[3]
            z = zp2.tile([128, 2, 512], F32, tag="z2")
            nc.tensor.matmul(
                z[:, 0, :], lhsJ[0:1, k0, :, :], rhsA[0:1, k0, :, :],
                start=True, stop=True, perf_mode=DR,
            )
            nc.tensor.matmul(
                z[:, 1, :], lhsJ[0:1, k1, :, :], rhsA[0:1, k1, :, :],
                start=True, stop=True, perf_mode=DR,
            )
            pos = fill[sign]
            t, slot = pos // 8, pos % 8
            tt, cap = tree[sign][t]
            nc.scalar.activation(tt[:, slot : slot + 2, :], z[:, :, :], AF.Relu)
            fill[sign] = pos + 2
            maybe_fold(sign)
        else:
            k = unit[2]
            z = zp1.tile([128, 512], F32, tag="z1")
            nc.tensor.matmul(
                z[:, :], lhsJ[0:1, k, :, :], rhsA[0:1, k, :, :],
                start=True, stop=True, perf_mode=DR,
            )
            acc_consume(sign, z[:, :])

    # cross-tile folds
    spart = {}
    for sign in (0, 1):
        f = folded[sign]
        for extra in f[1:]:
            nc.vector.tensor_add(f[0], f[0], extra)
        if f:
            spart[sign] = f[0]

    pos_parts = [p for p in (accs.get(("V", 1)), spart.get(1)) if p is not None]
    neg_parts = [p for p in (accs.get(("V", 0)), spart.get(0)) if p is not None]

    # combine: e = sum(pos) - sum(neg) + a2b
    e = work.tile([128, 512], F32, tag="ecomb", name=f"ecomb_{lay}")
    if pos_parts:
        nc.scalar.activation(e[:, :], pos_parts[0], AF.Identity, bias=float(a2b))
        rest_pos, rest_neg = pos_parts[1:], neg_parts
    else:
        nc.scalar.activation(
            e[:, :], neg_parts[0], AF.Identity, bias=float(a2b), scale=-1.0
        )
        rest_pos, rest_neg = [], neg_parts[1:]
    for p in rest_pos:
        nc.vector.tensor_add(e[:, :], e[:, :], p)
    for p in rest_neg:
        nc.vector.tensor_sub(e[:, :], e[:, :], p)

    # leaky relu, mask, exp(+rowsum)
    lr = work.tile([128, 512], F32, tag="lr", name=f"lr_{lay}")
    nc.vector.scalar_tensor_tensor(
        lr[:, :], e[:, :], NEG_SLOPE, e[:, :], op0=ALU.mult, op1=ALU.max
    )
    tm = work.tile([128, 512], F32, tag="tm", name=f"tm_{lay}")
    nc.vector.scalar_tensor_tensor(
        tm[:, :], lr[:, :], MASK_OFF, adjt[:, :], op0=ALU.add, op1=ALU.mult
    )
    E = work.tile([128, 512], BF16, tag="E", name=f"E_{lay}")
    sE = work.tile([128, 1], F32, tag="sE", name=f"sE_{lay}")
    nc.scalar.activation(
        E[:, :], tm[:, :], AF.Exp, bias=cst["moff"][:, :], accum_out=sE[:, :]
    )
    return E, sE


def _emit_ET(nc, pools, cst, lay, E):
    """Transpose E via PE (idle at the layer tail) + Scalar psum->sbuf copy."""
    work, mp = pools["work"], pools["mp"]
    ET = work.tile([128, NJC, 128], BF16, tag="ET", name=f"ET_{lay}")
    for jb in range(NJC):
        pt = mp.tile([128, 512], BF16, tag="mm", name=f"etp{jb}_{lay}")
        nc.tensor.transpose(
            pt[:, :128], E[:, jb * 128 : (jb + 1) * 128], cst["identt"][:, :]
        )
        nc.scalar.activation(ET[:, jb, :], pt[:, :128], AF.Copy, bias=0.0)
    return ET


def _emit_agg(nc, pools, lay, ET, h_tiles):
    mp = pools["mp"]
    pu = mp.tile([128, 512], F32, tag="mm", name=f"U_{lay}")
    for jb in range(NJC):
        nc.tensor.matmul(
            pu[:, :MEM], ET[:, jb, :], h_tiles[:, jb, 0:MEM],
            start=(jb == 0), stop=(jb == NJC - 1),
        )
    return pu


def _emit_sum_partial(nc, pools, cst, lay, sE):
    work, mp = pools["work"], pools["mp"]
    sEb = work.tile([128, 1], BF16, tag="sEb", name=f"sEb_{lay}")
    nc.vector.tensor_copy(sEb[:, :], sE[:, :])
    ps = mp.tile([128, 512], F32, tag="mm", name=f"sS_{lay}")
    nc.tensor.matmul(
        ps[:1, :1], sEb[:, 0:1], cst["onest"][:, 0:1], start=True, stop=True
    )
    sp = work.tile([1, 1], BF16, tag="sp", name=f"sp_{lay}")
    nc.vector.tensor_copy(sp[:, :], ps[:1, :1])
    return sp


def _build(p_pos, a2b, debug, dbg_taps=False):
    sched, counts = _consume_assignment(p_pos)
    nc = bacc.Bacc(
        "TRN2",
        target_bir_lowering=False,
        debug=debug,
        num_devices=NCORES,
    )
    d_dbg = {}
    if dbg_taps:
        for nm, shp in [
            ("dbg_E1", [128, 512]), ("dbg_E2", [128, 512]),
            ("dbg_U1b", [128, 300]),
        ]:
            d_dbg[nm] = nc.dram_tensor(nm, shp, BF16, kind="ExternalOutput")

    d_fT = nc.dram_tensor("fT", [128, 4 * N], BF16, kind="ExternalInput")
    d_fTo = nc.dram_tensor("fTo", [128, 4 * P], BF16, kind="ExternalInput")
    d_adj = nc.dram_tensor("adjm", [P, N], F32, kind="ExternalInput")
    d_w0 = nc.dram_tensor("w0b", [128, 4 * 300], BF16, kind="ExternalInput")
    d_w1 = nc.dram_tensor("w1b", [128, 3 * 300], BF16, kind="ExternalInput")
    d_wsi = nc.dram_tensor("wsib", [128, 4 * 64], BF16, kind="ExternalInput")
    d_csi = nc.dram_tensor("csic", [64, 1], F32, kind="ExternalInput")
    d_a1J = nc.dram_tensor("a1Jb", [128, 3 * 64], BF16, kind="ExternalInput")
    d_a1bc = nc.dram_tensor("a1bc", [64, 1], F32, kind="ExternalInput")
    d_b0c = nc.dram_tensor("b0c", [128, 3], F32, kind="ExternalInput")
    d_wsi2 = nc.dram_tensor("wsi2b", [128, 3 * 64], BF16, kind="ExternalInput")
    d_wsj2 = nc.dram_tensor("wsj2b", [128, 3 * 64], BF16, kind="ExternalInput")
    d_csi2 = nc.dram_tensor("csi2c", [64, 1], F32, kind="ExternalInput")
    d_csj2 = nc.dram_tensor("csj2c", [64, 1], F32, kind="ExternalInput")
    d_b1B = nc.dram_tensor("b1B", [128, 300], BF16, kind="ExternalInput")
    d_ones = nc.dram_tensor("onesb", [1, HID * 512], FP8, kind="ExternalInput")
    d_id = nc.dram_tensor("ident", [128, 128], BF16, kind="ExternalInput")
    d_outU = nc.dram_tensor("outU", [P, 300], F32, kind="ExternalOutput")
    d_outS = nc.dram_tensor("outS", [1, 1], F32, kind="ExternalOutput")

    with tile.TileContext(nc) as tc:
        with (
            tc.tile_pool(name="const", bufs=1) as const,
            tc.tile_pool(name="work", bufs=1) as work,
            tc.tile_pool(name="mp", bufs=2, space="PSUM") as mp,
            tc.tile_pool(name="zp2", bufs=2, space="PSUM") as zp2,
            tc.tile_pool(name="zp1", bufs=2, space="PSUM") as zp1,
            tc.tile_pool(name="dram", bufs=1, space="DRAM") as dram,
        ):
            pools = {"work": work, "mp": mp, "zp2": zp2, "zp1": zp1}

            # ---- const loads, ordered by first use; big ones chunked so
            # compute starts before the full load lands ----
            wsit = const.tile([128, 4, 64], BF16, tag="wsit")
            nc.sync.dma_start(wsit[:, :, :], d_wsi[:, :])
            fTo = const.tile([128, 4, 128], BF16, tag="fTo")
            nc.scalar.dma_start(fTo[:, :, :], d_fTo[:, :])
            csic = const.tile([64, 1], F32, tag="csic")
            nc.scalar.dma_start(csic[:, :], d_csi[:, :])
            w0t = const.tile([128, 4, 300], BF16, tag="w0t")
            fT = const.tile([128, 4, 512], BF16, tag="fT")
            for kt in range(4):
                [nc.sync, nc.scalar][kt % 2].dma_start(
                    w0t[:, kt, :], d_w0[:, kt * 300 : (kt + 1) * 300]
                )
                [nc.scalar, nc.sync][kt % 2].dma_start(
                    fT[:, kt, :], d_fT[:, kt * 512 : (kt + 1) * 512]
                )
            b0ct = const.tile([128, 3], F32, tag="b0ct")
            nc.gpsimd.dma_start(b0ct[:, :], d_b0c[:, :])
            a1Jt = const.tile([128, 3, 64], BF16, tag="a1Jt")
            nc.gpsimd.dma_start(a1Jt[:, :, :], d_a1J[:, :])
            a1bct = const.tile([64, 1], F32, tag="a1bct")
            nc.gpsimd.dma_start(a1bct[:, :], d_a1bc[:, :])
            adjt = const.tile([128, 512], F32, tag="adjt")
            nc.gpsimd.dma_start(adjt[:, :], d_adj[:, :])
            w1t = const.tile([128, 3, 300], BF16, tag="w1t")
            nc.gpsimd.dma_start(w1t[:, :, :], d_w1[:, :])
            wsi2t = const.tile([128, 3, 64], BF16, tag="wsi2t")
            nc.gpsimd.dma_start(wsi2t[:, :, :], d_wsi2[:, :])
            wsj2t = const.tile([128, 3, 64], BF16, tag="wsj2t")
            nc.gpsimd.dma_start(wsj2t[:, :, :], d_wsj2[:, :])
            csi2c = const.tile([64, 1], F32, tag="csi2c")
            nc.gpsimd.dma_start(csi2c[:, :], d_csi2[:, :])
            csj2c = const.tile([64, 1], F32, tag="csj2c")
            nc.gpsimd.dma_start(csj2c[:, :], d_csj2[:, :])
            b1Bt = const.tile([128, 300], BF16, tag="b1Bt")
            nc.gpsimd.dma_start(b1Bt[:, :], d_b1B[:, :])
            identt = const.tile([128, 128], BF16, tag="identt")
            nc.gpsimd.dma_start(identt[:, :], d_id[:, :])
            onest = const.tile([128, 128], BF16, tag="onest")
            nc.vector.memset(onest[:, :], 1.0)
            moff = const.tile([128, 1], F32, tag="moff")
            nc.vector.memset(moff[:, :], -MASK_OFF)
            zrow = const.tile([1, 128], BF16, tag="zrow")
            nc.vector.memset(zrow[:, :], 0.0)
            cst = dict(onest=onest, moff=moff, identt=identt, zrow=zrow)

            # DoubleRow fp8 operands: per k, lhsT = [siT_k(128) | ones(128)]
            # and rhs = [ones(512) | sjT_k(512)] packed on one partition
            lhsJ = work.tile([1, HID, 2, 128], FP8, tag="lhsJ")
            rhsA = work.tile([1, HID, 2, 512], FP8, tag="rhsA")
            nc.sync.dma_start(
                out=lhsJ[0:1, :, 1, :], in_=d_ones[0:1, 0 : HID * 128]
            )
            nc.sync.dma_start(out=rhsA[0:1, :, 0, :], in_=d_ones[0:1, :])

            # ================= LAYER 1 =================
            # siT1[k, j'] = wsi^T fTo + csi (host-folded; independent of hT1)
            siT1 = work.tile([64, 128], FP8, tag="siT1")
            ps = mp.tile([128, 512], F32, tag="mm", name="siT1p")
            for kt in range(4):
                nc.tensor.matmul(
                    ps[:64, :128], wsit[:, kt, :], fTo[:, kt, :],
                    start=(kt == 0), stop=(kt == 3),
                )
            nc.scalar.activation(
                siT1[:, :], ps[:64, :128], AF.Identity, bias=csic[:, :]
            )
            nc.scalar.dma_start(out=lhsJ[0:1, :, 0, :], in_=siT1[:, :])

            # hT1[m', n] = w0^T fT + b0, kt-outer so chunks start early
            hT1 = work.tile([128, 3, 512], BF16, tag="hT1")
            pms = [
                mp.tile([128, 512], F32, tag="mm", name="hT1m0"),
                mp.tile([128, 512], F32, tag="mm", name="hT1m1"),
                zp1.tile([128, 512], F32, tag="z1", name="hT1m2"),
            ]
            for kt in range(4):
                for mc in range(3):
                    msz, mo = MC[mc], mc * 128
                    nc.tensor.matmul(
                        pms[mc][:msz, :], w0t[:, kt, mo : mo + msz], fT[:, kt, :],
                        start=(kt == 0), stop=(kt == 3),
                    )
            for mc in range(3):
                nc.scalar.activation(
                    hT1[: MC[mc], mc, :], pms[mc][: MC[mc], :], AF.Identity,
                    bias=b0ct[: MC[mc], mc : mc + 1],
                )

            # sjT1[k, j] = a1J~^T hT1 + a1b~
            sjT1 = work.tile([64, 512], FP8, tag="sjT1")
            ps = mp.tile([128, 512], F32, tag="mm", name="sjT1p")
            for kt in range(3):
                nc.tensor.matmul(
                    ps[:64, :], a1Jt[: MC[kt], kt, :], hT1[: MC[kt], kt, :],
                    start=(kt == 0), stop=(kt == 2),
                )
            nc.scalar.activation(
                sjT1[:, :], ps[:64, :], AF.Identity, bias=a1bct[:, :]
            )
            nc.scalar.dma_start(out=rhsA[0:1, :, 1, :], in_=sjT1[:, :])

            # h1[j, m] tiles via XBAR transposes (chunk 2 padded 44->48 rows;
            # cols 300:304 never read)
            h1 = work.tile([128, NJC, 304], BF16, tag="h1")
            for jb in range(NJC):
                for mc in range(3):
                    mp_, mo = (128 if mc < 2 else 48), mc * 128
                    nc.sync.dma_start_transpose(
                        out=h1[:, jb, mo : mo + mp_],
                        in_=hT1[:mp_, mc, jb * 128 : (jb + 1) * 128],
                    )

            E1, sE1 = _emit_scores(
                nc, pools, cst, 0, lhsJ, rhsA, sched, counts, a2b, adjt
            )
            if dbg_taps:
                nc.sync.dma_start(out=d_dbg["dbg_E1"][:, :], in_=E1[:, :])
            sp1 = _emit_sum_partial(nc, pools, cst, 0, sE1)
            ET1 = _emit_ET(nc, pools, cst, 0, E1)
            pu1 = _emit_agg(nc, pools, 0, ET1, h1)
            U1b = work.tile([128, 300], BF16, tag="U1b")
            for mc in range(3):
                msz, mo = MC[mc], mc * 128
                nc.scalar.activation(
                    U1b[:, mo : mo + msz], pu1[:, mo : mo + msz],
                    AF.Copy, bias=0.0,
                )
            if dbg_taps:
                nc.sync.dma_start(out=d_dbg["dbg_U1b"][:, :], in_=U1b[:, 0:MEM])

            # U1T tiles [m-part, 128 i] via PE transpose (PE idle at tail;
            # rows 44:128 of chunk 2 are garbage and never read)
            U1T = work.tile([128, 3, 128], BF16, tag="U1T")
            for mc in range(3):
                msz = MC[mc]
                pt = mp.tile([128, 512], BF16, tag="mm", name=f"u1tp{mc}")
                nc.tensor.transpose(
                    pt[:msz, :128], U1b[:, mc * 128 : mc * 128 + msz],
                    identt[:, :],
                )
                nc.scalar.activation(
                    U1T[:msz, mc, :], pt[:msz, :128], AF.Copy, bias=0.0
                )

            # si2 raw from own U1T: si2o[i',k] = sum_m U1T[m,i'] wsi2[m,k]
            ps2 = mp.tile([128, 512], F32, tag="mm", name="si2p")
            for kt in range(3):
                nc.tensor.matmul(
                    ps2[:128, :64], U1T[: MC[kt], kt, :], wsi2t[: MC[kt], kt, :],
                    start=(kt == 0), stop=(kt == 2),
                )
            si2o = work.tile([128, 128], BF16, tag="si2o")
            nc.vector.tensor_copy(si2o[:, 0:64], ps2[:128, :64])
            siT2r = work.tile([128, 128], BF16, tag="siT2r")
            nc.scalar.dma_start_transpose(out=siT2r[:, :], in_=si2o[:, :])
            siT2 = work.tile([64, 128], FP8, tag="siT2")

            # gather payload: U1T chunks + S1 partial
            ccin = dram.tile([301, 128], BF16, tag="ccin")
            ccout = dram.tile([4 * 301, 128], BF16, tag="ccout")
            dma_engs = [nc.sync, nc.scalar, nc.gpsimd]
            for mc in range(3):
                msz, mo = MC[mc], mc * 128
                dma_engs[mc % 2].dma_start(
                    out=ccin[mo : mo + msz, :], in_=U1T[:msz, mc, :]
                )
            nc.sync.dma_start(out=ccin[300:301, :], in_=cst["zrow"][:, :])
            nc.sync.dma_start(out=ccin[300:301, 0:1], in_=sp1[:, :])

            nc.gpsimd.collective_compute(
                "AllGather",
                ALU.bypass,
                replica_groups=GROUPS,
                ins=[ccin.opt()],
                outs=[ccout.opt()],
            )

            # ---- post-gather: S1, gathered U1T ----
            sS4 = work.tile([4, 1], BF16, tag="sS4")
            nc.sync.dma_start(
                out=sS4[:, :], in_=ccout[300 : 4 * 301 : 301, 0:1]
            )
            psS = mp.tile([128, 512], F32, tag="mm", name="psS1")
            nc.tensor.matmul(
                psS[:128, 0:1], onest[0:4, :], sS4[:, :], start=True, stop=True
            )
            rS1 = work.tile([128, 1], F32, tag="rS1")
            nc.vector.reciprocal(rS1[:, :], psS[:128, 0:1])

            U1Tg = work.tile([128, 3, 512], BF16, tag="U1Tg")
            for s in range(4):
                nc.sync.dma_start(
                    out=U1Tg[:, 0:2, s * 128 : (s + 1) * 128],
                    in_=ccout[s * 301 : s * 301 + 256, :].rearrange(
                        "(m p) c -> p m c", m=2
                    ),
                )
                nc.scalar.dma_start(
                    out=U1Tg[:44, 2, s * 128 : (s + 1) * 128],
                    in_=ccout[s * 301 + 256 : s * 301 + 300, :],
                )

            # ================= LAYER 2 =================
            # sjT2 = (wsj2^T U1Tg) * rS1 + csj2   (all folds host-side)
            sjT2 = work.tile([64, 512], FP8, tag="sjT2")
            ps = mp.tile([128, 512], F32, tag="mm", name="sjT2p")
            for kt in range(3):
                nc.tensor.matmul(
                    ps[:64, :], wsj2t[: MC[kt], kt, :], U1Tg[: MC[kt], kt, :],
                    start=(kt == 0), stop=(kt == 2),
                )
            nc.scalar.activation(
                sjT2[:, :], ps[:64, :], AF.Identity,
                bias=csj2c[:, :], scale=rS1[:64, :],
            )
            nc.sync.dma_start(out=rhsA[0:1, :, 1, :], in_=sjT2[:, :])

            # siT2: raw value comes from the core's OWN U1T (pre-gather!);
            # only the rS1 scale + flatten wait on the collective. Emitted
            # here but siT2p/XBAR run as soon as U1T exists.
            nc.scalar.activation(
                siT2[:, :], siT2r[:64, :], AF.Identity,
                bias=csi2c[:, :], scale=rS1[:64, :],
            )
            nc.scalar.dma_start(out=lhsJ[0:1, :, 0, :], in_=siT2[:, :])

            E2, sE2 = _emit_scores(
                nc, pools, cst, 1, lhsJ, rhsA, sched, counts, a2b, adjt
            )
            # h2 raw = w1^T U1Tg (scale+bias applied later on [j,m] tiles)
            h2Traw = work.tile([128, 3, 512], BF16, tag="h2Traw")
            for mc in range(3):
                msz, mo = MC[mc], mc * 128
                ps = mp.tile([128, 512], F32, tag="mm", name=f"h2T{mc}")
                for kt in range(3):
                    nc.tensor.matmul(
                        ps[:msz, :],
                        w1t[: MC[kt], kt, mo : mo + msz],
                        U1Tg[: MC[kt], kt, :],
                        start=(kt == 0), stop=(kt == 2),
                    )
                nc.scalar.activation(
                    h2Traw[:msz, mc, :], ps[:msz, :], AF.Copy, bias=0.0
                )
            h2r = work.tile([128, NJC, 304], BF16, tag="h2r")
            for jb in range(NJC):
                for mc in range(3):
                    mp_, mo = (128 if mc < 2 else 48), mc * 128
                    nc.sync.dma_start_transpose(
                        out=h2r[:, jb, mo : mo + mp_],
                        in_=h2Traw[:mp_, mc, jb * 128 : (jb + 1) * 128],
                    )
            # h2sc = h2r * rS1 + b1 (DVE; Pool cannot run TensorScalarPtr)
            h2sc = work.tile([128, NJC, 304], BF16, tag="h2sc")
            for jb in range(NJC):
                nc.vector.scalar_tensor_tensor(
                    h2sc[:, jb, 0:MEM], h2r[:, jb, 0:MEM], rS1[:, :],
                    b1Bt[:, :], op0=ALU.mult, op1=ALU.add,
                )

            if dbg_taps:
                nc.sync.dma_start(out=d_dbg["dbg_E2"][:, :], in_=E2[:, :])
            sp2 = _emit_sum_partial(nc, pools, cst, 1, sE2)
            sp2f = work.tile([1, 1], F32, tag="sp2f")
            nc.vector.tensor_copy(sp2f[:, :], sp2[:, :])
            nc.scalar.dma_start(out=d_outS[:, :], in_=sp2f[:, :])
            ET2 = _emit_ET(nc, pools, cst, 1, E2)
            pu2 = _emit_agg(nc, pools, 1, ET2, h2sc)
            stout = work.tile([128, 300], F32, tag="stout")
            nc.scalar.activation(stout[:, :], pu2[:, :MEM], AF.Copy, bias=0.0)
            nc.sync.dma_start(out=d_outU[:, 0:150], in_=stout[:, 0:150])
            nc.scalar.dma_start(out=d_outU[:, 150:300], in_=stout[:, 150:300])

    nc.compile()
    return nc


_CACHE = {}


def _get_program(p_pos, a2b, debug=False, dbg_taps=False):
    key = (p_pos, float(a2b), debug, dbg_taps)
    if key not in _CACHE:
        _CACHE[key] = _build(p_pos, float(a2b), debug, dbg_taps=dbg_taps)
    return _CACHE[key]


def _pack_tiles(arr, nkt):
    """(rows, w) -> (128, nkt*w): row t*128+p lands at [p, t*w:(t+1)*w],
    zero-padding rows to nkt*128."""
    rows, w = arr.shape
    padded = np.zeros((nkt * 128, w), np.float32)
    padded[:rows] = arr
    return np.ascontiguousarray(
        padded.reshape(nkt, 128, w).transpose(1, 0, 2).reshape(128, nkt * w)
    )


def _prep_inputs(feature, adj, w0, b0, w1, b1, a1_w, a1_b, a2_w, a2_b):
    """Host-side packing: dtype casts, |a2| fold, sign sort, weight folds,
    shard slices."""
    bf = ml_dtypes.bfloat16
    a2 = np.asarray(a2_w, np.float32).reshape(-1)
    order = np.argsort((a2 < 0).astype(np.int32), kind="stable")
    p_pos = int((a2 >= 0).sum())
    absa2 = np.abs(a2[order])
    a1s = np.asarray(a1_w, np.float32)[:, order] * absa2[None, :]  # (600, 64)
    a1bs = np.asarray(a1_b, np.float32)[order] * absa2  # (64,)

    a1J = _pack_tiles(a1s[MEM:], 3).astype(bf)
    a1bc = a1bs[:, None].astype(np.float32)

    w0f = np.asarray(w0, np.float32)
    w1f = np.asarray(w1, np.float32)
    b0f = np.asarray(b0, np.float32)
    b1f = np.asarray(b1, np.float32)
    w0b = _pack_tiles(w0f, 4).astype(bf)
    w1b = _pack_tiles(w1f, 3).astype(bf)
    wsi = w0f @ a1s[:MEM]  # (512, 64)
    wsib = _pack_tiles(wsi, 4).astype(bf)
    csi = (b0f @ a1s[:MEM])[:, None].astype(np.float32)
    # layer-2 folds: everything linear in gathered U1^T
    wsi2b = _pack_tiles(w1f @ a1s[:MEM], 3).astype(bf)
    wsj2b = _pack_tiles(w1f @ a1s[MEM:], 3).astype(bf)
    csi2 = (b1f @ a1s[:MEM])[:, None].astype(np.float32)
    csj2 = (b1f @ a1s[MEM:] + a1bs)[:, None].astype(np.float32)
    b1B = np.broadcast_to(b1f[None, :], (128, MEM)).astype(bf).copy()
    b0c = np.zeros((128, 3), np.float32)
    for mc in range(3):
        b0c[: MC[mc], mc] = b0f[mc * 128 : mc * 128 + MC[mc]]
    onesb = np.ones((1, HID * 512), np.float32).astype(ml_dtypes.float8_e4m3fn)
    ident = np.eye(128, dtype=np.float32).astype(bf)

    featT = [np.asarray(feature[b], np.float32).T for b in range(B)]
    fTb = [_pack_tiles(featT[b], 4).astype(bf) for b in range(B)]
    adjf = np.asarray(adj, np.float32)
    in_maps = []
    for c in range(NCORES):
        b, i0 = c // 4, 128 * (c % 4)
        fTo = _pack_tiles(featT[b][:, i0 : i0 + P], 4).astype(bf)
        adjm = np.ascontiguousarray(adjf[b][i0 : i0 + P, :])
        in_maps.append(
            {
                "fT": fTb[b],
                "fTo": fTo,
                "adjm": adjm,
                "w0b": w0b,
                "w1b": w1b,
                "wsib": wsib,
                "csic": csi,
                "a1Jb": a1J,
                "a1bc": a1bc,
                "b0c": b0c,
                "wsi2b": wsi2b,
                "wsj2b": wsj2b,
                "csi2c": csi2,
                "csj2c": csj2,
                "b1B": b1B,
                "onesb": onesb,
                "ident": ident,
            }
        )
    a2b = float(np.asarray(a2_b, np.float32).reshape(-1)[0])
    return in_maps, p_pos, a2b


def kernel(feature, adj, w0, b0, w1, b1, a1_w, a1_b, a2_w, a2_b, _trace=False):
    in_maps, p_pos, a2b = _prep_inputs(
        feature, adj, w0, b0, w1, b1, a1_w, a1_b, a2_w, a2_b
    )
    nc = _get_program(p_pos, a2b, debug=False)
    res = run_bass_kernel_spmd(
        nc, in_maps, core_ids=list(range(NCORES)), trace=_trace
    )
    out = np.zeros((B, N, MEM), np.float32)
    for b in range(B):
        s = sum(
            float(np.asarray(res.results[4 * b + g]["outS"], np.float32)[0, 0])
            for g in range(4)
        )
        for g in range(4):
            u = np.asarray(res.results[4 * b + g]["outU"], np.float32)
            out[b, 128 * g : 128 * (g + 1), :] = u / s
    kernel._last_exec_time_ns = res.exec_time_ns
    kernel._last_profile = res.profile_json
    return out
